# revision 15
# baseline (speedup 1.0000x reference)
"""Neural CDE (RK4, piecewise-constant path derivative) Trainium2 kernel.

Strategy: pure batch parallelism, B=128 -> 16 per core across 8 cores.
Per core, the state is kept feature-major in "split form": a [128, 16] SBUF
tile whose semantic value is top[64] + bottom[64]. This lets the x-contraction
of the einsum land as a free-dim reduce and removes all fold ops from the
recurrence (the L1 weight matrix is stored row-duplicated so the matmul folds
the halves for free).

Per RK4 stage:
  MM1 (W1 split, K=128) -> relu(+b1) -> MM2 -> relu(+b2) ->
  b3 seed matmul + 8 accumulating W3-chunk matmuls (x-major column permute)
  -> tanh -> elementwise * [dt*v spread | -0.001*dt*sum(v)] -> strided reduce
  -> scalar_tensor_tensor updates for the RK4 combination.
"""

import os
import sys
import tempfile
from contextlib import ExitStack

import numpy as np
import ml_dtypes

sys.path.insert(0, "/opt/trn_rl_repo")

import concourse.bass as bass
import concourse.tile as tile
from concourse import bacc
from concourse import mybir
from concourse.bass_utils import run_bass_kernel_spmd

B, L, X, Z, H = 128, 512, 16, 64, 128
NCORES = 8
BPC = B // NCORES  # 16
DT = 0.1
F32 = mybir.dt.float32
AF = mybir.ActivationFunctionType
OP = mybir.AluOpType

# x-major permutation: psum position (p, c) holds original W3 column z*16+x
# with x = 2c + (p>=64), z = p%64
_p = np.arange(128)
_c = np.arange(8)
ORIG_COL = (_p[None, :] % 64) * 16 + 2 * _c[:, None] + (_p[None, :] // 64)  # [8,128]


def build_nc(l_steps=L, mm_bf16=True, kout=16):
    mmdt = mybir.dt.bfloat16 if mm_bf16 else F32
    npmm = ml_dtypes.bfloat16 if mm_bf16 else np.float32
    assert l_steps % kout == 0

    nc = bacc.Bacc("TRN2")

    # ---- DRAM parameters (per core) ----
    dp = nc.declare_dram_parameter
    vsmall = dp("vsmall", [l_steps, 256], F32, isOutput=False).ap()  # dt*v, x-major
    svd = dp("svd", [l_steps, 16], F32, isOutput=False).ap()  # -0.001*dt*sum_x v
    # packed so each matmul's SBUF inputs arrive via a single DMA (LDWEIGHTS
    # can only carry one DMA wait)
    wmm_d = dp("wmm", [128, 1280], mmdt, isOutput=False).ap()  # [w1s | w2 | w3x]
    b3sel_d = dp("b3sel", [8, 256], mmdt, isOutput=False).ap()  # [b3row | sel8]
    b1_d = dp("b1c", [128, 1], F32, isOutput=False).ap()
    b2_d = dp("b2c", [128, 1], F32, isOutput=False).ap()
    wi1x_d = dp("wi1x", [16, 144], F32, isOutput=False).ap()  # [wi1 | x0t]
    wi2_d = dp("wi2", [128, 128], F32, isOutput=False).ap()
    wi3_d = dp("wi3", [128, 64], F32, isOutput=False).ap()
    bi1_d = dp("bi1", [128, 1], F32, isOutput=False).ap()
    bi2_d = dp("bi2", [128, 1], F32, isOutput=False).ap()
    bi3_d = dp("bi3", [64, 1], F32, isOutput=False).ap()
    # split-form state per step; host folds top+bottom halves
    zall = dp("zall", [l_steps, 128, BPC], F32, isOutput=True).ap()

    with tile.TileContext(nc) as tc, ExitStack() as ctx:
        singles = ctx.enter_context(tc.tile_pool(name="singles", bufs=1))
        wfp = ctx.enter_context(tc.tile_pool(name="wfp", bufs=4))
        gep = ctx.enter_context(tc.tile_pool(name="gep", bufs=6))
        mp = ctx.enter_context(tc.tile_pool(name="mp", bufs=3))
        qp = ctx.enter_context(tc.tile_pool(name="qp", bufs=8))
        hp = ctx.enter_context(tc.tile_pool(name="hp", bufs=4))
        zbp = ctx.enter_context(tc.tile_pool(name="zbp", bufs=4))
        kp = ctx.enter_context(tc.tile_pool(name="kp", bufs=4))
        ph1p = ctx.enter_context(tc.tile_pool(name="ph1p", bufs=2, space="PSUM"))
        ph2p = ctx.enter_context(tc.tile_pool(name="ph2p", bufs=2, space="PSUM"))
        gpp = ctx.enter_context(tc.tile_pool(name="gpp", bufs=2, space="PSUM"))

        dma = nc.sync.dma_start

        # ---- load constants into SBUF ----
        def load(pool, ap, dtype=None):
            t = pool.tile(list(ap.shape), dtype or ap.dtype, tag=ap.tensor.name)
            dma(out=t[:], in_=ap)
            return t

        wmm = load(singles, wmm_d)
        w1s = wmm[:, 0:128]
        w2 = wmm[:, 128:256]
        w3x = wmm[:, 256:1280]
        b3sel = load(singles, b3sel_d)
        b3row = b3sel[:, 0:128]
        sel8 = b3sel[:, 128:256]
        b1c = load(singles, b1_d)
        b2c = load(singles, b2_d)
        wi1x = load(singles, wi1x_d)
        wi1 = wi1x[:, 0:128]
        x0t = wi1x[:, 128:144]
        wi2 = load(singles, wi2_d)
        wi3 = load(singles, wi3_d)
        bi1 = load(singles, bi1_d)
        bi2 = load(singles, bi2_d)
        bi3 = load(singles, bi3_d)

        # ---- init MLP (fp32): z0 = mlp(x(t0)) ----
        ph_i1 = ph1p.tile([128, BPC], F32, tag="ph1")
        nc.tensor.matmul(ph_i1[:], wi1, x0t, start=True, stop=True)
        hi1 = singles.tile([128, BPC], F32, tag="hi1")
        nc.scalar.activation(hi1[:], ph_i1[:], AF.Relu, bias=bi1[:])
        ph_i2 = ph2p.tile([128, BPC], F32, tag="ph2")
        nc.tensor.matmul(ph_i2[:], wi2[:], hi1[:], start=True, stop=True)
        hi2 = singles.tile([128, BPC], F32, tag="hi2")
        nc.scalar.activation(hi2[:], ph_i2[:], AF.Relu, bias=bi2[:])
        ph_i3 = ph1p.tile([64, BPC], F32, tag="ph1")
        nc.tensor.matmul(ph_i3[:], wi3[:], hi2[:], start=True, stop=True)

        # state tile for step 0: split form [z0; 0], plus bf16 copy for MM1
        ge_cur = gep.tile([128, 144], F32, tag="ge")
        nc.vector.tensor_scalar_add(ge_cur[0:64, 128:144], ph_i3[:], bi3[:])
        nc.vector.memset(ge_cur[64:128, 128:144], 0.0)
        zsb_cur = zbp.tile([128, BPC], mmdt, tag="zsb")
        nc.vector.tensor_copy(out=zsb_cur[:], in_=ge_cur[:, 128:144])

        stage_scale = [0.5, 0.5, 1.0]  # q_s multiplier for next stage input

        for t in range(l_steps):
            # build wf [128, 144] by replication-DMA from vsmall/svd
            wf = wfp.tile([128, 144], F32, tag="wf")
            vbase = vsmall[t]
            for half in range(2):
                src = bass.AP(
                    tensor=vbase.tensor,
                    offset=vbase.offset + 16 * half,
                    ap=[[0, 64], [32, 8], [1, 16]],
                )
                dst = wf[64 * half:64 * (half + 1), 0:128].rearrange(
                    "p (c j) -> p c j", j=16
                )
                dma(out=dst, in_=src)
            sbase = svd[t]
            src = bass.AP(tensor=sbase.tensor, offset=sbase.offset,
                          ap=[[0, 128], [1, 16]])
            dma(out=wf[:, 128:144], in_=src)

            # output: split-form state at start of step t (host folds halves)
            dma(out=zall[t], in_=ge_cur[:, 128:144])

            qs = []
            ge_s, zsb_s = ge_cur, zsb_cur
            ge_next = None
            kacc12 = kacc123 = pfin = None
            for s in range(4):
                ph1 = ph1p.tile([128, BPC], F32, tag="ph1")
                nc.tensor.matmul(ph1[:], w1s, zsb_s[:], start=True, stop=True)
                h1 = hp.tile([128, BPC], mmdt, tag="h1")
                nc.scalar.activation(h1[:], ph1[:], AF.Relu, bias=b1c[:])
                ph2 = ph2p.tile([128, BPC], F32, tag="ph2")
                nc.tensor.matmul(ph2[:], w2, h1[:], start=True, stop=True)
                h2 = hp.tile([128, BPC], mmdt, tag="h2")
                nc.scalar.activation(h2[:], ph2[:], AF.Relu, bias=b2c[:])

                gp = gpp.tile([128, 128], F32, tag="gp")
                nc.tensor.matmul(gp[:], b3row, sel8, start=True, stop=False,
                                 skip_group_check=True)
                for c in range(8):
                    nc.tensor.matmul(
                        gp[:, c * 16:(c + 1) * 16],
                        w3x[:, c * 128:(c + 1) * 128], h2[:],
                        start=False, stop=(c == 7), skip_group_check=True,
                    )
                nc.scalar.activation(ge_s[:, 0:128], gp[:], AF.Tanh, bias=0.0)
                m = mp.tile([128, 144], F32, tag="m")
                nc.vector.tensor_tensor(out=m[:], in0=ge_s[:, 0:144], in1=wf[:],
                                        op=OP.mult)
                q = qp.tile([128, BPC], F32, tag="q")
                nc.vector.tensor_reduce(
                    out=q[:], in_=m[:].rearrange("p (c j) -> p j c", j=16),
                    axis=mybir.AxisListType.X, op=OP.add,
                )
                qs.append(q)

                # RK4 bookkeeping
                if s < 3:
                    ge_n = gep.tile([128, 144], F32, tag="ge")
                    zsb_n = zbp.tile([128, BPC], mmdt, tag="zsb")
                    # critical path: bf16 next-stage input on vector
                    nc.vector.scalar_tensor_tensor(
                        out=zsb_n[:], in0=q[:], scalar=stage_scale[s],
                        in1=ge_cur[:, 128:144], op0=OP.mult, op1=OP.add,
                    )
                    # f32 slot copy on gpsimd (feeds m-mul of next stage + folds)
                    nc.vector.scalar_tensor_tensor(
                        out=ge_n[:, 128:144], in0=q[:], scalar=stage_scale[s],
                        in1=ge_cur[:, 128:144], op0=OP.mult, op1=OP.add,
                    )
                    ge_s, zsb_s = ge_n, zsb_n
                if s == 1:
                    kacc12 = kp.tile([128, BPC], F32, tag="k")
                    nc.vector.scalar_tensor_tensor(
                        out=kacc12[:], in0=qs[1][:], scalar=2.0, in1=qs[0][:],
                        op0=OP.mult, op1=OP.add,
                    )
                elif s == 2:
                    kacc123 = kp.tile([128, BPC], F32, tag="k")
                    nc.vector.scalar_tensor_tensor(
                        out=kacc123[:], in0=qs[2][:], scalar=2.0, in1=kacc12[:],
                        op0=OP.mult, op1=OP.add,
                    )
                    pfin = kp.tile([128, BPC], F32, tag="k")
                    nc.vector.scalar_tensor_tensor(
                        out=pfin[:], in0=kacc123[:], scalar=1.0 / 6.0,
                        in1=ge_cur[:, 128:144], op0=OP.mult, op1=OP.add,
                    )
                elif s == 3:
                    ge_next = gep.tile([128, 144], F32, tag="ge")
                    zsb_next = zbp.tile([128, BPC], mmdt, tag="zsb")
                    nc.vector.scalar_tensor_tensor(
                        out=zsb_next[:], in0=q[:], scalar=1.0 / 6.0, in1=pfin[:],
                        op0=OP.mult, op1=OP.add,
                    )
                    nc.vector.scalar_tensor_tensor(
                        out=ge_next[:, 128:144], in0=q[:], scalar=1.0 / 6.0,
                        in1=pfin[:], op0=OP.mult, op1=OP.add,
                    )
            ge_cur, zsb_cur = ge_next, zsb_next

    nc.compile()
    return nc


def _prep_inputs(t, x, dyn_w1, dyn_b1, dyn_w2, dyn_b2, dyn_w3, dyn_b3,
                 init_w1, init_b1, init_w2, init_b2, init_w3, init_b3,
                 mm_bf16=True, l_steps=L):
    npmm = ml_dtypes.bfloat16 if mm_bf16 else np.float32
    x = np.asarray(x, dtype=np.float32)
    x_aug = np.concatenate([x, x[:, -1:]], axis=1)
    v = (x_aug[:, 1:] - x_aug[:, :-1]) / DT  # [B, L, X]
    sv = v.sum(-1)  # [B, L]

    w1s = np.concatenate([dyn_w1, dyn_w1], axis=0).astype(npmm)
    w2 = np.asarray(dyn_w2, dtype=npmm)
    w3x = np.empty((H, 1024), dtype=npmm)
    for c in range(8):
        w3x[:, c * 128:(c + 1) * 128] = dyn_w3[:, ORIG_COL[c]].astype(npmm)
    b3row = np.asarray(dyn_b3)[ORIG_COL].astype(npmm)
    sel8 = np.zeros((8, 128), dtype=npmm)
    for k in range(8):
        sel8[k, k * 16:(k + 1) * 16] = 1.0

    wmm = np.concatenate([w1s, w2, w3x], axis=1)        # [128, 1280]
    b3sel = np.concatenate([b3row, sel8], axis=1)       # [8, 256]

    shared = dict(
        wmm=np.ascontiguousarray(wmm), b3sel=np.ascontiguousarray(b3sel),
        b1c=np.asarray(dyn_b1, np.float32).reshape(128, 1),
        b2c=np.asarray(dyn_b2, np.float32).reshape(128, 1),
        wi2=np.asarray(init_w2, np.float32),
        wi3=np.asarray(init_w3, np.float32),
        bi1=np.asarray(init_b1, np.float32).reshape(128, 1),
        bi2=np.asarray(init_b2, np.float32).reshape(128, 1),
        bi3=np.asarray(init_b3, np.float32).reshape(64, 1),
    )
    wi1 = np.asarray(init_w1, np.float32)

    in_maps = []
    for core in range(NCORES):
        sl = slice(core * BPC, (core + 1) * BPC)
        vb = v[sl, :l_steps]            # [BPC, l, X]
        svb = sv[sl, :l_steps]          # [BPC, l]
        # vsmall[t, x*16+j] = DT * v[j, t, x]
        vsm = (DT * vb.transpose(1, 2, 0)).reshape(l_steps, 256).astype(np.float32)
        svdc = (-0.001 * DT * svb.T).astype(np.float32)  # [l, BPC]
        x0tc = x[sl, 0, :].T.astype(np.float32)          # [X, BPC]
        wi1x = np.concatenate([wi1, x0tc], axis=1)       # [16, 144]
        m = dict(shared)
        m.update(vsmall=np.ascontiguousarray(vsm), svd=np.ascontiguousarray(svdc),
                 wi1x=np.ascontiguousarray(wi1x))
        in_maps.append(m)
    return in_maps


_NC_CACHE = {}


MM_BF16 = False


def kernel_traced(trace=False, mm_bf16=MM_BF16, **inputs):
    key = (L, mm_bf16)
    if key not in _NC_CACHE:
        _NC_CACHE[key] = build_nc(L, mm_bf16=mm_bf16)
    nc = _NC_CACHE[key]
    in_maps = _prep_inputs(**inputs, mm_bf16=mm_bf16, l_steps=L)
    res = run_bass_kernel_spmd(nc, in_maps, list(range(NCORES)), trace=trace)
    out = np.empty((B, L, Z), dtype=np.float32)
    for core in range(NCORES):
        zall = res.results[core]["zall"]  # [L, 128, BPC] split form
        zf = zall[:, :Z] + zall[:, Z:]
        out[core * BPC:(core + 1) * BPC] = zf.transpose(2, 0, 1)
    return out, res


def kernel(**inputs):
    return kernel_traced(trace=False, **inputs)[0]
